# revision 14
# baseline (speedup 1.0000x reference)
"""NVFP4-style activation quantizer (nn_ActQuantizer) on 8 TRN2 NeuronCores.

Self-contained: hardcodes shapes/sharding for x of shape (2, 2048, 4096) f32.
Data-parallel: the flat 16.8M-element tensor is split into 8 contiguous
2,097,152-element shards (each [128 partitions x 16384]), one per core.
Groups of 16 contiguous elements stay within a partition row.

Algorithm (validated bit-exact vs the fp16/e4m3 reference semantics on HW):
  amax   = group abs-max                  (tensor_reduce, abs=True)
  scale  = round_to_e4m3(amax)            ((bits + 0x7FFFF) >>20 <<20, clamp)
  f      = fp16(x * (6/scale))            (f32 mult, fp16 RNE writeback)
  ql     = min(fp16(f*(1+2^-11) + 768) - 768, 1)   (magic-add rounds to 0.5)
  r1     = bits(f) + 0x100 >>9 <<9        (round to 1 mantissa bit, ties away)
  q      = |f| >= 1 ? r1 : ql             (copy_predicated)
  out    = q * (scale/6)                  (fp16 x f32-bcast -> f32)

Engine split: DVE reduce/mult/select, ScalarE the affine fp16 ops,
GPSIMD the r1 integer ops, DMA double-buffered via tile pools.
"""
import sys

sys.path.insert(0, "/opt/trn_rl_repo")

import numpy as np

import concourse.bass as bass
import concourse.bacc as bacc
import concourse.mybir as mybir
from concourse import tile
from concourse.bass_utils import run_bass_kernel_spmd

AF = mybir.ActivationFunctionType
ALU = mybir.AluOpType

N_CORES = 8
FULL_SHAPE = (2, 2048, 4096)
TOTAL = 2 * 2048 * 4096            # 16,777,216
PER_CORE = TOTAL // N_CORES        # 2,097,152
P = 128
FD = PER_CORE // P                 # 16384 free elems per partition
TILE_SIZES = [4096, 4096, 4096, 4096]
assert sum(TILE_SIZES) == FD

S0 = float(np.float32(1.0) + np.float32(2.0 ** -11))

_cached_nc = None


def build_nc() -> bass.Bass:
    nc = bacc.Bacc("TRN2", target_bir_lowering=False, debug=False)
    x = nc.dram_tensor("x", [P, FD], mybir.dt.float32, kind="ExternalInput")
    out = nc.dram_tensor("out", [P, FD], mybir.dt.float32, kind="ExternalOutput")

    with tile.TileContext(nc) as tc:
        with tc.tile_pool(name="xin", bufs=2) as xin_pool, \
             tc.tile_pool(name="yout", bufs=2) as yout_pool, \
             tc.tile_pool(name="work", bufs=3) as work, \
             tc.tile_pool(name="small", bufs=3) as small:
            off = 0
            for t, FT in enumerate(TILE_SIZES):
                GT = FT // 16
                sl = slice(off, off + FT)
                off += FT
                xt = xin_pool.tile([P, FT], mybir.dt.float32, tag="x")
                if t == 0:
                    # first fill: split across both HWDGE engines for 2x BW
                    h = FT // 2
                    nc.sync.dma_start(out=xt[:, :h], in_=x[:, sl.start:sl.start + h])
                    nc.scalar.dma_start(out=xt[:, h:], in_=x[:, sl.start + h:sl.stop])
                else:
                    nc.sync.dma_start(out=xt[:], in_=x[:, sl])

                # group amax (DVE, 1x)
                am = small.tile([P, GT], mybir.dt.float32, tag="am")
                nc.vector.tensor_reduce(
                    am[:], xt[:].rearrange("p (g s) -> p g s", s=16),
                    axis=mybir.AxisListType.X, op=ALU.max,
                    apply_absolute_value=True,
                )
                # e4m3 round bit trick + clamp -> scale (DVE smalls)
                sr = small.tile([P, GT], mybir.dt.float32, tag="sr")
                nc.vector.tensor_scalar(
                    sr[:].bitcast(mybir.dt.int32), am[:].bitcast(mybir.dt.int32),
                    0x7FFFF, None, ALU.add,
                )
                nc.vector.tensor_scalar(
                    sr[:].bitcast(mybir.dt.int32), sr[:].bitcast(mybir.dt.int32),
                    20, 20, ALU.logical_shift_right, ALU.logical_shift_left,
                )
                # r6 = 6/scale (DVE); o32 = scale/6 (ACT small)
                r6 = small.tile([P, GT], mybir.dt.float32, tag="r6")
                nc.vector.reciprocal(r6[:], sr[:])
                nc.vector.tensor_scalar_mul(r6[:], r6[:], 6.0)
                o32 = small.tile([P, GT], mybir.dt.float32, tag="o32")
                nc.scalar.activation(o32[:], sr[:], AF.Copy, scale=1.0 / 6.0)

                # f = fp16(x * r6bcast) (DVE, 1x)
                ft = work.tile([P, FT], mybir.dt.float16, tag="f")
                nc.vector.tensor_tensor(
                    ft[:].rearrange("p (g s) -> p g s", s=16),
                    xt[:].rearrange("p (g s) -> p g s", s=16),
                    r6[:].unsqueeze(2).broadcast_to((P, GT, 16)),
                    ALU.mult,
                )
                # mabs = |f| and u5 = fp16(f*s0 + 768), both on ACT
                m = work.tile([P, FT], mybir.dt.float16, tag="m")
                nc.scalar.activation(m[:], ft[:], AF.Abs)
                q = work.tile([P, FT], mybir.dt.float16, tag="q")
                nc.scalar.activation(q[:], ft[:], AF.Copy, bias=768.0, scale=S0)
                # DVE work independent of ACT first:
                # r1 = (bits(f) + 0x100) >>9 <<9 in place over f (DVE 4x)
                nc.vector.tensor_scalar(
                    ft[:].bitcast(mybir.dt.int16), ft[:].bitcast(mybir.dt.int16),
                    0x100, None, ALU.add,
                )
                nc.vector.tensor_scalar(
                    ft[:].bitcast(mybir.dt.int16), ft[:].bitcast(mybir.dt.int16),
                    9, 9, ALU.logical_shift_right, ALU.logical_shift_left,
                )
                # c = (mabs >= 1.0) in place (DVE 4x)
                nc.vector.tensor_scalar(
                    m[:].bitcast(mybir.dt.int16), m[:], 1.0, None, ALU.is_ge,
                )
                # ql = min(u5 - 768, 1.0) in place (DVE fp16 4x)
                nc.vector.tensor_scalar(q[:], q[:], 768.0, 1.0,
                                        ALU.subtract, ALU.min)
                # select: q = where(c, r1, ql) (DVE, 1x)
                nc.vector.copy_predicated(q[:], m[:].bitcast(mybir.dt.int16), ft[:])
                # y32 = q * o32bcast (DVE, 1x, f32 out)
                yt = yout_pool.tile([P, FT], mybir.dt.float32, tag="y")
                nc.vector.tensor_tensor(
                    yt[:].rearrange("p (g s) -> p g s", s=16),
                    q[:].rearrange("p (g s) -> p g s", s=16),
                    o32[:].unsqueeze(2).broadcast_to((P, GT, 16)),
                    ALU.mult,
                )
                if t == len(TILE_SIZES) - 1:
                    h = FT // 2
                    nc.sync.dma_start(out=out[:, sl.start:sl.start + h], in_=yt[:, :h])
                    nc.scalar.dma_start(out=out[:, sl.start + h:sl.stop], in_=yt[:, h:])
                else:
                    nc.sync.dma_start(out=out[:, sl], in_=yt[:])
    nc.compile()
    return nc


def _get_nc() -> bass.Bass:
    global _cached_nc
    if _cached_nc is None:
        _cached_nc = build_nc()
    return _cached_nc


def run(x: np.ndarray, trace: bool = False, **kw):
    """Shard, run SPMD on 8 cores, gather. Returns (out_full, BassKernelResults)."""
    x_flat = np.ascontiguousarray(np.asarray(x, dtype=np.float32)).reshape(-1)
    in_maps = [
        {"x": x_flat[i * PER_CORE:(i + 1) * PER_CORE].reshape(P, FD)}
        for i in range(N_CORES)
    ]
    nc = _get_nc()
    res = run_bass_kernel_spmd(nc, in_maps, core_ids=list(range(N_CORES)),
                               trace=trace, **kw)
    out = np.empty(TOTAL, dtype=np.float32)
    for i in range(N_CORES):
        out[i * PER_CORE:(i + 1) * PER_CORE] = res.results[i]["out"].reshape(-1)
    return out.reshape(FULL_SHAPE), res


def kernel(x: np.ndarray) -> np.ndarray:
    out, _ = run(x, trace=False)
    return out
